# revision 3
# baseline (speedup 1.0000x reference)
"""CharLSTM Trainium2 kernel, single-core 3-phase design (zero collectives).

Phase 1: layer-1 scan with full Wh[0] resident in SBUF (16.8MB) and the
  input projection folded into a one-hot matmul against E1 = embed@Wx[0]+b[0].
  h1T(t) written to HBM each step.
Phase 2: G2 = hs1 @ Wx[1] + b[1] as a For_i GEMM over timesteps.
Phase 3: layer-2 scan with full Wh[1] resident, G2 streamed from HBM,
  out(t) = h2(t) @ W_out fused in-step.

All three phases are For_i hardware loops (compact program). Gate column
order is [i|f|o|g] blocks of 1024 so chunk c of 512 cols has a uniform
activation (c<6: sigmoid, else tanh).
"""
import os
import numpy as np

V, H, L, B, T = 128, 1024, 2, 64, 512
G = 4 * H
KT = H // 128     # 8 contraction tiles
NC8 = G // 512    # 8 N-chunks per gate row


def _build_nc():
    import concourse.mybir as mybir
    from concourse import bacc
    from concourse.tile import TileContext
    from concourse.masks import make_identity
    from concourse.bass import ts, ds

    f32 = mybir.dt.float32
    AF = mybir.ActivationFunctionType

    nc = bacc.Bacc("TRN2", target_bir_lowering=False, name="charlstm2")

    d_wh1 = nc.dram_tensor("wh1", [KT, 128, G], f32, kind="ExternalInput")
    d_wx2 = nc.dram_tensor("wx2", [KT, 128, G], f32, kind="ExternalInput")
    d_wh2 = nc.dram_tensor("wh2", [KT, 128, G], f32, kind="ExternalInput")
    d_e1 = nc.dram_tensor("e1", [128, G], f32, kind="ExternalInput")
    d_b2 = nc.dram_tensor("b2", [1, G], f32, kind="ExternalInput")
    d_wout = nc.dram_tensor("wout", [KT, 128, V], f32, kind="ExternalInput")
    d_oh = nc.dram_tensor("oh", [T * 128, B], f32, kind="ExternalInput")
    d_out = nc.dram_tensor("out", [T * B, V], f32, kind="ExternalOutput")
    d_h1T = nc.dram_tensor("h1T", [T * 128, KT * B], f32)   # internal
    T4 = T // 4 if T % 4 == 0 else T
    NQ = T // T4
    # G2 split into quarters to stay under the 256MB DRAM scratch page
    d_g2 = [nc.dram_tensor(f"g2_{q}", [T4 * B, G], f32) for q in range(NQ)]

    def scan(tc, wh_sb, e1_or_none, ident, h_T, c_sb, gx_dram, wout_sb,
             wpool, gpspool, tpspool, opspool, ohpool, ts, t0, span):
        """One For_i scan loop. Layer 1 when e1_or_none is set (one-hot
        input proj, h1T written to HBM); layer 2 otherwise (G2 streamed,
        out-projection fused)."""
        layer1 = e1_or_none is not None

        def body(i):
            ifo = wpool.tile([64, 3 * H], f32, tag="ifo", name="ifo", bufs=1)
            gg = wpool.tile([64, H], f32, tag="gg", name="gg", bufs=1)
            if layer1:
                oh = ohpool.tile([128, B], f32, tag="oh", name="oh")
                nc.sync.dma_start(oh[:], d_oh[ds(i * 128 + t0 * 128, 128), :])
            else:
                gx = wpool.tile([64, G], f32, tag="gx", name="gx", bufs=1)
                nc.sync.dma_start(gx[:, 0:2048],
                                  gx_dram[ts(i, B), 0:2048])
                nc.sync.dma_start(gx[:, 2048:4096],
                                  gx_dram[ts(i, B), 2048:4096])
            for c in range(NC8):
                g_ps = gpspool.tile([64, 512], f32, tag="g", name="g_ps")
                if layer1:
                    nc.tensor.matmul(g_ps[:], oh[:],
                                     e1_or_none[:, c * 512:(c + 1) * 512],
                                     start=True, stop=False)
                for kt in range(KT):
                    nc.tensor.matmul(
                        g_ps[:],
                        h_T[:, kt * B:(kt + 1) * B],
                        wh_sb[:, kt * G + c * 512: kt * G + (c + 1) * 512],
                        start=(not layer1 and kt == 0),
                        stop=(kt == KT - 1))
                if not layer1:
                    nc.vector.tensor_add(g_ps[:], g_ps[:],
                                         gx[:, c * 512:(c + 1) * 512])
                if c < 6:
                    nc.scalar.activation(ifo[:, c * 512:(c + 1) * 512],
                                         g_ps[:], AF.Sigmoid)
                else:
                    nc.scalar.activation(gg[:, (c - 6) * 512:(c - 5) * 512],
                                         g_ps[:], AF.Tanh)
            t1 = wpool.tile([64, H], f32, tag="t1", name="t1", bufs=1)
            t2 = wpool.tile([64, H], f32, tag="t2", name="t2", bufs=1)
            nc.vector.tensor_mul(t1[:], ifo[:, 0:H], gg[:])
            nc.vector.tensor_mul(t2[:], ifo[:, H:2 * H], c_sb[:])
            nc.vector.tensor_add(c_sb[:], t1[:], t2[:])
            tch = wpool.tile([64, H], f32, tag="tch", name="tch", bufs=1)
            nc.scalar.activation(tch[:], c_sb[:], AF.Tanh)
            h_sb = wpool.tile([64, H], f32, tag="h", name="h_sb", bufs=1)
            nc.vector.tensor_mul(h_sb[:], ifo[:, 2 * H:3 * H], tch[:])
            pT = tpspool.tile([128, KT * B], f32, tag="pT", name="pT")
            for kt in range(KT):
                nc.tensor.transpose(pT[:, kt * B:(kt + 1) * B],
                                    h_sb[:, kt * 128:(kt + 1) * 128],
                                    ident[0:64, 0:64])
            nc.vector.tensor_copy(h_T[:], pT[:])
            if layer1:
                nc.sync.dma_start(d_h1T[ds(i * 128 + t0 * 128, 128), :],
                                  h_T[:])
            else:
                o_ps = opspool.tile([64, V], f32, tag="o", name="o_ps")
                for kt in range(KT):
                    nc.tensor.matmul(o_ps[:], h_T[:, kt * B:(kt + 1) * B],
                                     wout_sb[:, kt * V:(kt + 1) * V],
                                     start=(kt == 0), stop=(kt == KT - 1))
                o_sb = wpool.tile([64, V], f32, tag="osb", name="o_sb")
                nc.vector.tensor_copy(o_sb[:], o_ps[:])
                nc.sync.dma_start(d_out[ds(i * B + t0 * B, B), :], o_sb[:])

        with tc.For_i(0, span, 1) as i:
            body(i)

    with TileContext(nc) as tc:
        with tc.tile_pool(name="gps", bufs=2, space="PSUM") as gpspool, \
             tc.tile_pool(name="tps", bufs=2, space="PSUM") as tpspool, \
             tc.tile_pool(name="ops", bufs=2, space="PSUM") as opspool, \
             tc.tile_pool(name="state", bufs=1) as spool, \
             tc.tile_pool(name="oh", bufs=2) as ohpool:

            ident = spool.tile([128, 128], f32, tag="ident", name="ident")
            make_identity(nc, ident[:])
            h_T = spool.tile([128, KT * B], f32, tag="hT", name="h_T")
            c_sb = spool.tile([64, H], f32, tag="c", name="c_sb")

            # ---- phase 1: layer-1 scan ----
            with tc.tile_pool(name="w1", bufs=1) as w1pool, \
                 tc.tile_pool(name="wk1", bufs=2) as wk1:
                wh1 = w1pool.tile([128, KT * G], f32, tag="wh1", name="wh1")
                e1 = w1pool.tile([128, G], f32, tag="e1", name="e1")
                for kt in range(KT):
                    nc.sync.dma_start(wh1[:, kt * G:(kt + 1) * G], d_wh1[kt])
                nc.sync.dma_start(e1[:], d_e1[:])
                nc.vector.memset(h_T[:], 0.0)
                nc.vector.memset(c_sb[:], 0.0)
                scan(tc, wh1, e1, ident, h_T, c_sb, None, None,
                     wk1, gpspool, tpspool, opspool, ohpool, ts, 0, T)

            # ---- phase 2: G2 = hs1 @ Wx2 + b2 ----
            with tc.tile_pool(name="w2", bufs=1) as w2pool, \
                 tc.tile_pool(name="wk2", bufs=2) as wk2:
                wx2 = w2pool.tile([128, KT * G], f32, tag="wx2", name="wx2")
                b2 = w2pool.tile([1, G], f32, tag="b2", name="b2")
                ones1 = w2pool.tile([1, B], f32, tag="ones1", name="ones1")
                for kt in range(KT):
                    nc.sync.dma_start(wx2[:, kt * G:(kt + 1) * G], d_wx2[kt])
                nc.sync.dma_start(b2[:], d_b2[:])
                nc.vector.memset(ones1[:], 1.0)

                def gbody(m, q):
                    lh = wk2.tile([128, KT * B], f32, tag="lh", name="lh")
                    nc.sync.dma_start(
                        lh[:], d_h1T[ds(m * 128 + q * T4 * 128, 128), :])
                    for c in range(NC8):
                        g_ps = gpspool.tile([64, 512], f32, tag="g",
                                            name="g_ps2")
                        nc.tensor.matmul(g_ps[:], ones1[0:1, :],
                                         b2[0:1, c * 512:(c + 1) * 512],
                                         start=True, stop=False)
                        for kt in range(KT):
                            nc.tensor.matmul(
                                g_ps[:], lh[:, kt * B:(kt + 1) * B],
                                wx2[:, kt * G + c * 512:kt * G + (c + 1) * 512],
                                start=False, stop=(kt == KT - 1))
                        gsb = wk2.tile([64, 512], f32, tag="gsb",
                                       name="gsb")
                        nc.vector.tensor_copy(gsb[:], g_ps[:])
                        nc.sync.dma_start(
                            d_g2q[ts(m, B), c * 512:(c + 1) * 512], gsb[:])

                for q in range(NQ):
                    d_g2q = d_g2[q]
                    with tc.For_i(0, T4, 1) as m:
                        gbody(m, q)

            # ---- phase 3: layer-2 scan ----
            with tc.tile_pool(name="w3", bufs=1) as w3pool, \
                 tc.tile_pool(name="wk3", bufs=2) as wk3:
                wh2 = w3pool.tile([128, KT * G], f32, tag="wh2", name="wh2")
                wout = w3pool.tile([128, KT * V], f32, tag="wout", name="wout")
                for kt in range(KT):
                    nc.sync.dma_start(wh2[:, kt * G:(kt + 1) * G], d_wh2[kt])
                    nc.sync.dma_start(wout[:, kt * V:(kt + 1) * V], d_wout[kt])
                nc.vector.memset(h_T[:], 0.0)
                nc.vector.memset(c_sb[:], 0.0)
                for q in range(NQ):
                    scan(tc, wh2, None, ident, h_T, c_sb, d_g2[q], wout,
                         wk3, gpspool, tpspool, opspool, ohpool, ts,
                         q * T4, T4)

    nc.compile()
    return nc


def _host_prep(idx, embed, Wx, Wh, b, W_out):
    idx = np.asarray(idx)
    embed = np.asarray(embed, np.float32)
    Wx = np.asarray(Wx, np.float32)
    Wh = np.asarray(Wh, np.float32)
    b = np.asarray(b, np.float32)
    W_out = np.asarray(W_out, np.float32)

    perm = np.concatenate([np.arange(g * H, (g + 1) * H)
                           for g in (0, 1, 3, 2)])   # [i|f|o|g]
    E1 = (embed @ Wx[0] + b[0])[:, perm]
    onehot = (idx.T[:, None, :] == np.arange(V, dtype=idx.dtype)[None, :, None])
    oh = np.ascontiguousarray(onehot.astype(np.float32).reshape(T * 128, B))

    return {
        "wh1": np.ascontiguousarray(Wh[0][:, perm].reshape(KT, 128, G)),
        "wx2": np.ascontiguousarray(Wx[1][:, perm].reshape(KT, 128, G)),
        "wh2": np.ascontiguousarray(Wh[1][:, perm].reshape(KT, 128, G)),
        "e1": np.ascontiguousarray(E1),
        "b2": np.ascontiguousarray(b[1][perm][None, :]),
        "wout": np.ascontiguousarray(W_out.reshape(KT, 128, V)),
        "oh": oh,
    }


_NC_CACHE = {}


def kernel(idx, embed, Wx, Wh, b, W_out):
    from concourse.bass_interp import get_hw_module
    from concourse.bass_utils import run_bass_kernel_spmd

    if "nc" not in _NC_CACHE:
        nc = _build_nc()
        nc.m = get_hw_module(nc.m)
        _NC_CACHE["nc"] = nc
    nc = _NC_CACHE["nc"]

    in_map = _host_prep(idx, embed, Wx, Wh, b, W_out)
    res = run_bass_kernel_spmd(nc, [in_map], core_ids=[0])
    _NC_CACHE["last_results"] = res
    out = res.results[0]["out"]
    return np.ascontiguousarray(
        out.reshape(T, B, V).transpose(1, 0, 2)).astype(np.float32)


# revision 4
# speedup vs baseline: 1.2207x; 1.2207x over previous
"""CharLSTM Trainium2 kernel, single-core 3-phase design (zero collectives).

Phase 1: layer-1 scan with full Wh[0] resident in SBUF (16.8MB) and the
  input projection folded into a one-hot matmul against E1 = embed@Wx[0]+b[0].
  h1T(t) written to HBM each step.
Phase 2: G2 = hs1 @ Wx[1] + b[1] as a For_i GEMM over timesteps.
Phase 3: layer-2 scan with full Wh[1] resident, G2 streamed from HBM,
  out(t) = h2(t) @ W_out fused in-step.

All three phases are For_i hardware loops (compact program). Gate column
order is [i|f|o|g] blocks of 1024 so chunk c of 512 cols has a uniform
activation (c<6: sigmoid, else tanh).
"""
import os
import numpy as np

V, H, L, B, T = 128, 1024, 2, 64, 512
G = 4 * H
KT = H // 128     # 8 contraction tiles
NC8 = G // 512    # 8 N-chunks per gate row


def _build_nc():
    import concourse.mybir as mybir
    from concourse import bacc
    from concourse.tile import TileContext
    from concourse.masks import make_identity
    from concourse.bass import ts, ds

    f32 = mybir.dt.float32
    AF = mybir.ActivationFunctionType

    nc = bacc.Bacc("TRN2", target_bir_lowering=False, name="charlstm2")

    d_wh1 = nc.dram_tensor("wh1", [KT, 128, G], f32, kind="ExternalInput")
    d_wx2 = nc.dram_tensor("wx2", [KT, 128, G], f32, kind="ExternalInput")
    d_wh2 = nc.dram_tensor("wh2", [KT, 128, G], f32, kind="ExternalInput")
    d_e1 = nc.dram_tensor("e1", [128, G], f32, kind="ExternalInput")
    d_b2 = nc.dram_tensor("b2", [1, G], f32, kind="ExternalInput")
    d_wout = nc.dram_tensor("wout", [KT, 128, V], f32, kind="ExternalInput")
    d_oh = nc.dram_tensor("oh", [T * 128, B], mybir.dt.uint8,
                          kind="ExternalInput")
    d_out = nc.dram_tensor("out", [T * B, V], f32, kind="ExternalOutput")
    d_h1T = nc.dram_tensor("h1T", [T * 128, KT * B], f32)   # internal
    T4 = T // 4 if T % 4 == 0 else T
    NQ = T // T4
    # G2 split into quarters to stay under the 256MB DRAM scratch page
    # paired layout: row = t*128 + half*64 + b, col = pair*512 + n
    d_g2 = [nc.dram_tensor(f"g2_{q}", [T4 * 128, G // 2], f32)
            for q in range(NQ)]

    def scan(tc, wh_sb, e1_or_none, ident, h_T, c_sb, gx_dram, wout_sb,
             wpool, gpspool, tpspool, opspool, ohpool, ts, t0, span):
        """One For_i scan loop. Layer 1 when e1_or_none is set (one-hot
        input proj, h1T written to HBM); layer 2 otherwise (G2 streamed,
        out-projection fused)."""
        layer1 = e1_or_none is not None

        def body(i):
            # paired layout: [128, 512] tiles, rows 0:64 = chunk 2p,
            # rows 64:128 = chunk 2p+1 (col-group packed matmuls)
            ifo = wpool.tile([128, 1536], f32, tag="ifo", name="ifo", bufs=1)
            gg = wpool.tile([128, 512], f32, tag="gg", name="gg", bufs=1)
            if layer1:
                oh8 = ohpool.tile([128, B], mybir.dt.uint8, tag="oh8",
                                  name="oh8")
                nc.sync.dma_start(oh8[:], d_oh[ds(i * 128 + t0 * 128, 128), :])
                oh = ohpool.tile([128, B], f32, tag="oh", name="oh")
                nc.vector.tensor_copy(oh[:], oh8[:])
            else:
                gx = wpool.tile([128, G // 2], f32, tag="gx", name="gx",
                                bufs=1)
                nc.sync.dma_start(gx[:], gx_dram[ts(i, 128), :])
            for p in range(NC8 // 2):
                g_ps = gpspool.tile([128, 512], f32, tag="g", name="g_ps")
                for half in range(2):
                    c = 2 * p + half
                    o_sl = g_ps[64 * half:64 * half + 64, :]
                    tp = (0, 64 * half)
                    if layer1:
                        nc.tensor.matmul(o_sl, oh[:],
                                         e1_or_none[:, c * 512:(c + 1) * 512],
                                         start=True, stop=False,
                                         tile_position=tp)
                    for kt in range(KT):
                        nc.tensor.matmul(
                            o_sl,
                            h_T[:, kt * B:(kt + 1) * B],
                            wh_sb[:, kt * G + c * 512: kt * G + (c + 1) * 512],
                            start=(not layer1 and kt == 0),
                            stop=(kt == KT - 1), tile_position=tp)
                if not layer1:
                    nc.vector.tensor_add(g_ps[:], g_ps[:],
                                         gx[:, p * 512:(p + 1) * 512])
                if p < 3:
                    nc.scalar.activation(ifo[:, p * 512:(p + 1) * 512],
                                         g_ps[:], AF.Sigmoid)
                else:
                    nc.scalar.activation(gg[:], g_ps[:], AF.Tanh)
            t1 = wpool.tile([128, 512], f32, tag="t1", name="t1", bufs=1)
            t2 = wpool.tile([128, 512], f32, tag="t2", name="t2", bufs=1)
            nc.vector.tensor_mul(t1[:], ifo[:, 0:512], gg[:])
            nc.vector.tensor_mul(t2[:], ifo[:, 512:1024], c_sb[:])
            nc.vector.tensor_add(c_sb[:], t1[:], t2[:])
            tch = wpool.tile([128, 512], f32, tag="tch", name="tch", bufs=1)
            nc.scalar.activation(tch[:], c_sb[:], AF.Tanh)
            h_sb = wpool.tile([128, 512], f32, tag="h", name="h_sb", bufs=1)
            nc.vector.tensor_mul(h_sb[:], ifo[:, 1024:1536], tch[:])
            # shift upper half down so all transposes read base partition 0
            h_hi = wpool.tile([64, 512], f32, tag="hhi", name="h_hi", bufs=1)
            nc.sync.dma_start(h_hi[:], h_sb[64:128, :])
            pT = tpspool.tile([128, KT * B], f32, tag="pT", name="pT")
            for kt in range(KT):
                half, cc = kt // 4, (kt % 4) * 128
                src_t = h_sb[0:64, cc:cc + 128] if half == 0 \
                    else h_hi[0:64, cc:cc + 128]
                nc.tensor.transpose(pT[:, kt * B:(kt + 1) * B], src_t,
                                    ident[0:64, 0:64])
            nc.vector.tensor_copy(h_T[:], pT[:])
            if layer1:
                nc.sync.dma_start(d_h1T[ds(i * 128 + t0 * 128, 128), :],
                                  h_T[:])
            else:
                o_ps = opspool.tile([64, V], f32, tag="o", name="o_ps")
                for kt in range(KT):
                    nc.tensor.matmul(o_ps[:], h_T[:, kt * B:(kt + 1) * B],
                                     wout_sb[:, kt * V:(kt + 1) * V],
                                     start=(kt == 0), stop=(kt == KT - 1))
                o_sb = wpool.tile([64, V], f32, tag="osb", name="o_sb")
                nc.vector.tensor_copy(o_sb[:], o_ps[:])
                nc.sync.dma_start(d_out[ds(i * B + t0 * B, B), :], o_sb[:])

        with tc.For_i(0, span, 1) as i:
            body(i)

    with TileContext(nc) as tc:
        with tc.tile_pool(name="gps", bufs=2, space="PSUM") as gpspool, \
             tc.tile_pool(name="tps", bufs=2, space="PSUM") as tpspool, \
             tc.tile_pool(name="ops", bufs=2, space="PSUM") as opspool, \
             tc.tile_pool(name="state", bufs=1) as spool, \
             tc.tile_pool(name="oh", bufs=2) as ohpool:

            ident = spool.tile([128, 128], f32, tag="ident", name="ident")
            make_identity(nc, ident[:])
            h_T = spool.tile([128, KT * B], f32, tag="hT", name="h_T")
            c_sb = spool.tile([128, 512], f32, tag="c", name="c_sb")

            # ---- phase 1: layer-1 scan ----
            with tc.tile_pool(name="w1", bufs=1) as w1pool, \
                 tc.tile_pool(name="wk1", bufs=2) as wk1:
                wh1 = w1pool.tile([128, KT * G], f32, tag="wh1", name="wh1")
                e1 = w1pool.tile([128, G], f32, tag="e1", name="e1")
                for kt in range(KT):
                    nc.sync.dma_start(wh1[:, kt * G:(kt + 1) * G], d_wh1[kt])
                nc.sync.dma_start(e1[:], d_e1[:])
                nc.vector.memset(h_T[:], 0.0)
                nc.vector.memset(c_sb[:], 0.0)
                scan(tc, wh1, e1, ident, h_T, c_sb, None, None,
                     wk1, gpspool, tpspool, opspool, ohpool, ts, 0, T)

            # ---- phase 2: G2 = hs1 @ Wx2 + b2 ----
            with tc.tile_pool(name="w2", bufs=1) as w2pool, \
                 tc.tile_pool(name="wk2", bufs=2) as wk2:
                wx2 = w2pool.tile([128, KT * G], f32, tag="wx2", name="wx2")
                b2 = w2pool.tile([1, G], f32, tag="b2", name="b2")
                ones1 = w2pool.tile([1, B], f32, tag="ones1", name="ones1")
                for kt in range(KT):
                    nc.sync.dma_start(wx2[:, kt * G:(kt + 1) * G], d_wx2[kt])
                nc.sync.dma_start(b2[:], d_b2[:])
                nc.vector.memset(ones1[:], 1.0)

                def gbody(m, q):
                    lh = wk2.tile([128, KT * B], f32, tag="lh", name="lh")
                    nc.sync.dma_start(
                        lh[:], d_h1T[ds(m * 128 + q * T4 * 128, 128), :])
                    for p in range(NC8 // 2):
                        g_ps = gpspool.tile([128, 512], f32, tag="g",
                                            name="g_ps2")
                        for half in range(2):
                            c = 2 * p + half
                            o_sl = g_ps[64 * half:64 * half + 64, :]
                            tp = (0, 64 * half)
                            nc.tensor.matmul(o_sl, ones1[0:1, :],
                                             b2[0:1, c * 512:(c + 1) * 512],
                                             start=True, stop=False,
                                             tile_position=tp)
                            for kt in range(KT):
                                nc.tensor.matmul(
                                    o_sl, lh[:, kt * B:(kt + 1) * B],
                                    wx2[:, kt * G + c * 512:
                                        kt * G + (c + 1) * 512],
                                    start=False, stop=(kt == KT - 1),
                                    tile_position=tp)
                        gsb = wk2.tile([128, 512], f32, tag="gsb",
                                       name="gsb")
                        nc.vector.tensor_copy(gsb[:], g_ps[:])
                        nc.sync.dma_start(
                            d_g2q[ts(m, 128), p * 512:(p + 1) * 512], gsb[:])

                for q in range(NQ):
                    d_g2q = d_g2[q]
                    with tc.For_i(0, T4, 1) as m:
                        gbody(m, q)

            # ---- phase 3: layer-2 scan ----
            with tc.tile_pool(name="w3", bufs=1) as w3pool, \
                 tc.tile_pool(name="wk3", bufs=2) as wk3:
                wh2 = w3pool.tile([128, KT * G], f32, tag="wh2", name="wh2")
                wout = w3pool.tile([128, KT * V], f32, tag="wout", name="wout")
                for kt in range(KT):
                    nc.sync.dma_start(wh2[:, kt * G:(kt + 1) * G], d_wh2[kt])
                    nc.sync.dma_start(wout[:, kt * V:(kt + 1) * V], d_wout[kt])
                nc.vector.memset(h_T[:], 0.0)
                nc.vector.memset(c_sb[:], 0.0)
                for q in range(NQ):
                    scan(tc, wh2, None, ident, h_T, c_sb, d_g2[q], wout,
                         wk3, gpspool, tpspool, opspool, ohpool, ts,
                         q * T4, T4)

    nc.compile()
    return nc


def _host_prep(idx, embed, Wx, Wh, b, W_out):
    idx = np.asarray(idx)
    embed = np.asarray(embed, np.float32)
    Wx = np.asarray(Wx, np.float32)
    Wh = np.asarray(Wh, np.float32)
    b = np.asarray(b, np.float32)
    W_out = np.asarray(W_out, np.float32)

    perm = np.concatenate([np.arange(g * H, (g + 1) * H)
                           for g in (0, 1, 3, 2)])   # [i|f|o|g]
    E1 = (embed @ Wx[0] + b[0])[:, perm]
    onehot = (idx.T[:, None, :] == np.arange(V, dtype=idx.dtype)[None, :, None])
    oh = np.ascontiguousarray(onehot.astype(np.uint8).reshape(T * 128, B))

    return {
        "wh1": np.ascontiguousarray(Wh[0][:, perm].reshape(KT, 128, G)),
        "wx2": np.ascontiguousarray(Wx[1][:, perm].reshape(KT, 128, G)),
        "wh2": np.ascontiguousarray(Wh[1][:, perm].reshape(KT, 128, G)),
        "e1": np.ascontiguousarray(E1),
        "b2": np.ascontiguousarray(b[1][perm][None, :]),
        "wout": np.ascontiguousarray(W_out.reshape(KT, 128, V)),
        "oh": oh,
    }


_NC_CACHE = {}


def kernel(idx, embed, Wx, Wh, b, W_out):
    from concourse.bass_interp import get_hw_module
    from concourse.bass_utils import run_bass_kernel_spmd

    if "nc" not in _NC_CACHE:
        nc = _build_nc()
        nc.m = get_hw_module(nc.m)
        _NC_CACHE["nc"] = nc
    nc = _NC_CACHE["nc"]

    in_map = _host_prep(idx, embed, Wx, Wh, b, W_out)
    res = run_bass_kernel_spmd(nc, [in_map], core_ids=[0])
    _NC_CACHE["last_results"] = res
    out = res.results[0]["out"]
    return np.ascontiguousarray(
        out.reshape(T, B, V).transpose(1, 0, 2)).astype(np.float32)
